# revision 1
# baseline (speedup 1.0000x reference)
# Circular convolution along channels == matmul with a circulant matrix:
#   y[r, n] = sum_k x[r, k] * W[(n - k) mod 2048],  W = W_first_col * W_second_col
# Shard rows (8*64*64 = 32768) across 8 NeuronCores; replicate the 2048x1536
# circulant matrix. Per core: [4096, 2048] @ [2048, 1536] fp16 matmul, fp32 out
# (fp16 runs at the same 1 cycle/row PE rate as bf16 but has 10 mantissa bits).
import numpy as np

IN_DIM = 2048
OUT_DIM = 1536
N_CORES = 8
ROWS = 8 * 64 * 64            # 32768
RPC = ROWS // N_CORES         # 4096 rows per core

P = 128                       # partitions
K_TILES = IN_DIM // P         # 16
N_TILE = 512                  # PSUM bank = 512 fp32
N_CHUNKS = OUT_DIM // N_TILE  # 3
ROW_TILE = 512                # rows per streamed x tile
N_ROW_TILES = RPC // ROW_TILE     # 8
RB_PER_TILE = ROW_TILE // P       # 4 row-blocks per x tile

_cache = {}


def _emit_body(nc, xpool, opool, pspool, wt, xT, y):
    import concourse.mybir as mybir

    for rt in range(N_ROW_TILES):
        xts = []
        for kt in range(K_TILES):
            xt_t = xpool.tile([P, ROW_TILE], mybir.dt.float16,
                              name=f"x{kt}_{rt}", tag=f"x{kt}")
            nc.sync.dma_start(
                xt_t[:],
                xT[kt * P:(kt + 1) * P, rt * ROW_TILE:(rt + 1) * ROW_TILE],
            )
            xts.append(xt_t)

        for rb in range(RB_PER_TILE):
            ps = pspool.tile([P, N_CHUNKS, N_TILE], mybir.dt.float32,
                             name=f"ps_{rt}_{rb}", tag="ps")
            for kt in range(K_TILES):
                lhsT = xts[kt][:, rb * P:(rb + 1) * P]
                for n in range(N_CHUNKS):
                    nc.tensor.matmul(
                        ps[:, n, :],
                        lhsT,
                        wt[(kt - 4 * n) % K_TILES][:],
                        start=(kt == 0),
                        stop=(kt == K_TILES - 1),
                    )
            ot = opool.tile([P, OUT_DIM], mybir.dt.float32,
                            name=f"o_{rt}_{rb}", tag="ot")
            for n in range(N_CHUNKS):
                nc.vector.tensor_copy(ot[:, n * N_TILE:(n + 1) * N_TILE],
                                      ps[:, n, :])
            row0 = rt * ROW_TILE + rb * P
            nc.sync.dma_start(y[row0:row0 + P, :], ot[:])


def _build(repeat=1):
    import contextlib

    import concourse.bass as bass
    import concourse.mybir as mybir
    import concourse.tile as tile
    from concourse import bacc

    nc = bacc.Bacc(
        "TRN2",
        target_bir_lowering=False,
        debug=False,
        enable_asserts=False,
        num_devices=N_CORES,
    )
    xT = nc.dram_tensor("xT", (IN_DIM, RPC), mybir.dt.float16, kind="ExternalInput")
    mm = nc.dram_tensor("mm", (IN_DIM, N_TILE), mybir.dt.float16, kind="ExternalInput")
    y = nc.dram_tensor("y", (RPC, OUT_DIM), mybir.dt.float32, kind="ExternalOutput")

    with tile.TileContext(nc) as tc:
        with (
            tc.tile_pool(name="w", bufs=1) as wpool,
            tc.tile_pool(name="x", bufs=3) as xpool,
            tc.tile_pool(name="o", bufs=3) as opool,
            tc.tile_pool(name="ps", bufs=2, space=bass.MemorySpace.PSUM) as pspool,
        ):
            # Resident circulant weights: only the FIRST 512 output columns
            # (16 k-tiles of [128, 512] fp16, 2 MB). Columns n+512 of the
            # circulant are k-rotations of columns n (M[k, n+512] =
            # M[(k-512) mod 2048, n]), and 512 = 4 k-tiles, so n-chunk c
            # reuses the same tiles at rotated index (kt - 4c) mod 16.
            # Preload split across the gpsimd/scalar DMA rings so it streams
            # concurrently with the x loads on the sync ring.
            wt = []
            for kt in range(K_TILES):
                w = wpool.tile([P, N_TILE], mybir.dt.float16,
                               name=f"w{kt}", tag=f"w{kt}")
                e = nc.gpsimd if kt % 2 == 0 else nc.scalar
                e.dma_start(w[:], mm[kt * P:(kt + 1) * P, :])
                wt.append(w)

            if repeat > 1:
                with tc.For_i(0, repeat, 1):
                    _emit_body(nc, xpool, opool, pspool, wt, xT, y)
            else:
                _emit_body(nc, xpool, opool, pspool, wt, xT, y)

    nc.compile()
    return nc


def kernel(x: np.ndarray, W_first_col: np.ndarray, W_second_col: np.ndarray) -> np.ndarray:
    from concourse import bass_utils

    W = (np.asarray(W_first_col, np.float32)
         * np.asarray(W_second_col, np.float32))[:IN_DIM]
    # circulant, first N_TILE output columns only: mmat[k, n] = W[(n - k) mod IN_DIM]
    # (columns n+512c are k-rotations of these; the kernel reindexes tiles)
    idx = (np.arange(N_TILE)[None, :] - np.arange(IN_DIM)[:, None]) % IN_DIM
    mmat = np.ascontiguousarray(W[idx]).astype(np.float16)

    xf = np.asarray(x, np.float32).reshape(ROWS, IN_DIM)
    in_maps = []
    for c in range(N_CORES):
        shard = xf[c * RPC:(c + 1) * RPC].astype(np.float16)
        xTc = np.ascontiguousarray(shard.T)  # [IN_DIM, RPC]
        in_maps.append({"xT": xTc, "mm": mmat})

    if "nc" not in _cache:
        _cache["nc"] = _build()
    try:
        res = bass_utils.run_bass_kernel_spmd(
            _cache["nc"], in_maps, core_ids=list(range(N_CORES))
        )
    except Exception:
        # transient device/exec failures usually clear on a retry
        res = bass_utils.run_bass_kernel_spmd(
            _cache["nc"], in_maps, core_ids=list(range(N_CORES))
        )
    out = np.concatenate([r["y"] for r in res.results], axis=0)
    return out.reshape(8, 64, 64, OUT_DIM)



# revision 3
# speedup vs baseline: 2.5939x; 2.5939x over previous
# Circular conv along channels, via CRT factorization of the circulant:
#   conv mod (z^2048-1) = CRT[ conv mod (z^1024-1), conv mod (z^1024+1) ]
# and the cyclic-1024 splits again into cyclic-512 + negacyclic-512. With
# folds  u = x_lo + x_hi, v = x_lo - x_hi,  uu = u_lo + u_hi, uv = u_lo - u_hi
# and scale factors baked into the weight matrices:
#   y[   0: 512] = uu@Mcc + uv@Mcv + (v@Mn)[:,   0: 512]
#   y[ 512:1024] = uu@Mcc - uv@Mcv + (v@Mn)[:, 512:1024]
#   y[1024:1536] = uu@Mcc + uv@Mcv - (v@Mn)[:,   0: 512]
# Matmul work per core: 2x [4096,512]@[512,512] + [4096,1024]@[1024,1024]
# = exactly half of the direct [4096,2048]@[2048,1536]. fp16 in / fp32 psum.
# Weight-stationary layout (computes y^T with rows streaming as the moving
# operand) so each 128x128 stationary block serves 2 back-to-back matmuls;
# host transposes the output back. ScalarE drains PSUM -> SBUF (fp16 cast),
# VectorE does folds + reconstruction, all at 2x_1P fp16 rate.
import numpy as np

IN_DIM = 2048
OUT_DIM = 1536
N_CORES = 8
ROWS = 8 * 64 * 64            # 32768
RPC = ROWS // N_CORES         # 4096 rows per core

P = 128                       # partitions
GROUP = 1024                  # rows per pipeline stage
N_GROUPS = RPC // GROUP       # 4
H = 512                       # one PSUM bank of fp32 / half a GROUP

_cache = {}


def _fold_mats(W_first_col, W_second_col):
    W = (np.asarray(W_first_col, np.float64)
         * np.asarray(W_second_col, np.float64))[:IN_DIM]
    Wu = W[:1024] + W[1024:]          # cyclic-1024 kernel
    Wv = W[:1024] - W[1024:]          # negacyclic-1024 kernel
    Wuu = Wu[:512] + Wu[512:]         # cyclic-512 kernel
    Wuv = Wu[:512] - Wu[512:]         # negacyclic-512 kernel
    k5 = np.arange(512)[:, None]
    n5 = np.arange(512)[None, :]
    k10 = np.arange(1024)[:, None]
    n10 = np.arange(1024)[None, :]
    # M[k, n] = w[(n-k) mod N] (cyclic) with sign flip for wrapped terms
    # (negacyclic); CRT halving factors 1/4 and 1/2 baked in.
    mcc = Wuu[(n5 - k5) % 512] * 0.25
    mcv = Wuv[(n5 - k5) % 512] * np.where(k5 <= n5, 0.25, -0.25)
    mn = Wv[(n10 - k10) % 1024] * np.where(k10 <= n10, 0.5, -0.5)
    return (np.ascontiguousarray(mcc).astype(np.float16),
            np.ascontiguousarray(mcv).astype(np.float16),
            np.ascontiguousarray(mn).astype(np.float16))


def make_in_maps(x, W_first_col, W_second_col):
    mcc, mcv, mn = _fold_mats(W_first_col, W_second_col)
    xf = np.asarray(x, np.float32).reshape(ROWS, IN_DIM).astype(np.float16)
    in_maps = []
    for c in range(N_CORES):
        xTc = np.ascontiguousarray(xf[c * RPC:(c + 1) * RPC].T)  # [IN_DIM, RPC]
        in_maps.append({"xT": xTc, "mcc": mcc, "mcv": mcv, "mn": mn})
    return in_maps


def _emit_body(nc, pools, wts, xT, yT):
    import concourse.mybir as mybir

    add = mybir.AluOpType.add
    sub = mybir.AluOpType.subtract
    wcc, wcv, wn = wts
    xpool, upool, fpool, spool, tpool, opool, pspool = pools

    def loads_folds(g):
        # Load/fold order is chosen so the first cyc-512 matmul only waits on
        # one quad of x tiles (~1 MB of DMA), not the whole 4 MB group: for
        # each q we load x[q], x[q+8], x[q+4], x[q+12] and immediately emit
        # u[q], u[q+4], uu[q], uv[q]. The v folds (only needed by the last
        # matmul phase) trail.
        xt = [None] * 16
        u = [None] * 8
        uu, uv, v = [], [], []

        def load(kt):
            t = xpool.tile([P, GROUP], mybir.dt.float16,
                           name=f"x{kt}_{g}", tag=f"x{kt}")
            nc.sync.dma_start(
                t[:], xT[kt * P:(kt + 1) * P, g * GROUP:(g + 1) * GROUP])
            xt[kt] = t

        def fold_u(kt):
            t = upool.tile([P, GROUP], mybir.dt.float16,
                           name=f"u{kt}_{g}", tag=f"u{kt}")
            nc.vector.tensor_tensor(t[:], xt[kt][:], xt[kt + 8][:], add)
            u[kt] = t

        for q in range(4):
            for kt in (q, q + 8, q + 4, q + 12):
                load(kt)
            fold_u(q)
            fold_u(q + 4)
            t = fpool.tile([P, GROUP], mybir.dt.float16,
                           name=f"uu{q}_{g}", tag=f"uu{q}")
            nc.vector.tensor_tensor(t[:], u[q][:], u[q + 4][:], add)
            uu.append(t)
            t = fpool.tile([P, GROUP], mybir.dt.float16,
                           name=f"uv{q}_{g}", tag=f"uv{q}")
            nc.vector.tensor_tensor(t[:], u[q][:], u[q + 4][:], sub)
            uv.append(t)
        for kt in range(8):
            t = fpool.tile([P, GROUP], mybir.dt.float16,
                           name=f"v{kt}_{g}", tag=f"v{kt}")
            nc.vector.tensor_tensor(t[:], xt[kt][:], xt[kt + 8][:], sub)
            v.append(t)
        return v, uu, uv

    def mm_phase(g, name, w, rhs_tiles, n_blocks, k_tiles):
        outs = []
        for nb in range(n_blocks):
            ps = pspool.tile([P, GROUP], mybir.dt.float32,
                             name=f"ps_{name}{nb}_{g}", tag="ps")
            for kt in range(k_tiles):
                lhsT = w[kt][:, nb * P:(nb + 1) * P]
                nc.tensor.matmul(ps[:, 0:H], lhsT, rhs_tiles[kt][:, 0:H],
                                 start=(kt == 0), stop=(kt == k_tiles - 1))
                nc.tensor.matmul(ps[:, H:2 * H], lhsT, rhs_tiles[kt][:, H:2 * H],
                                 start=(kt == 0), stop=(kt == k_tiles - 1))
            st = spool.tile([P, GROUP], mybir.dt.float16,
                            name=f"{name}{nb}_{g}", tag=f"{name}{nb}")
            nc.scalar.copy(st[:], ps[:])
            outs.append(st)
        return outs

    def recon_store(g, yc, yv, nn):
        for j in range(4):
            t1 = tpool.tile([P, GROUP], mybir.dt.float16,
                            name=f"t1_{j}_{g}", tag="t1")
            t2 = tpool.tile([P, GROUP], mybir.dt.float16,
                            name=f"t2_{j}_{g}", tag="t2")
            nc.vector.tensor_tensor(t1[:], yc[j][:], yv[j][:], add)
            nc.vector.tensor_tensor(t2[:], yc[j][:], yv[j][:], sub)
            for name, a, b, op, r0 in (
                ("o0", t1, nn[j], add, j * P),
                ("o1", t2, nn[j + 4], add, 512 + j * P),
                ("o2", t1, nn[j], sub, 1024 + j * P),
            ):
                o = opool.tile([P, GROUP], mybir.dt.float16,
                               name=f"{name}_{j}_{g}", tag=name)
                nc.vector.tensor_tensor(o[:], a[:], b[:], op)
                nc.scalar.dma_start(
                    yT[r0:r0 + P, g * GROUP:(g + 1) * GROUP], o[:])

    cur = loads_folds(0)
    for g in range(N_GROUPS):
        v, uu, uv = cur
        yc = mm_phase(g, "yc", wcc, uu, 4, 4)
        yv = mm_phase(g, "yv", wcv, uv, 4, 4)
        nn = mm_phase(g, "nn", wn, v, 8, 8)
        if g + 1 < N_GROUPS:
            cur = loads_folds(g + 1)
        recon_store(g, yc, yv, nn)


def _build(repeat=1):
    import concourse.bass as bass
    import concourse.mybir as mybir
    import concourse.tile as tile
    from concourse import bacc

    nc = bacc.Bacc(
        "TRN2",
        target_bir_lowering=False,
        debug=False,
        enable_asserts=False,
        num_devices=N_CORES,
    )
    xT = nc.dram_tensor("xT", (IN_DIM, RPC), mybir.dt.float16, kind="ExternalInput")
    mcc = nc.dram_tensor("mcc", (512, 512), mybir.dt.float16, kind="ExternalInput")
    mcv = nc.dram_tensor("mcv", (512, 512), mybir.dt.float16, kind="ExternalInput")
    mn = nc.dram_tensor("mn", (1024, 1024), mybir.dt.float16, kind="ExternalInput")
    yT = nc.dram_tensor("yT", (OUT_DIM, RPC), mybir.dt.float16, kind="ExternalOutput")

    with tile.TileContext(nc) as tc:
        with (
            tc.tile_pool(name="w", bufs=1) as wpool,
            tc.tile_pool(name="x", bufs=1) as xpool,
            tc.tile_pool(name="u", bufs=1) as upool,
            tc.tile_pool(name="f", bufs=2) as fpool,
            tc.tile_pool(name="s", bufs=1) as spool,
            tc.tile_pool(name="t", bufs=2) as tpool,
            tc.tile_pool(name="o", bufs=2) as opool,
            tc.tile_pool(name="ps", bufs=3, space=bass.MemorySpace.PSUM) as pspool,
        ):
            # Resident weights: split the preload across the scalar/sync DMA
            # rings so it streams alongside the first x loads.
            wcc, wcv, wn = [], [], []
            for kt in range(4):
                t = wpool.tile([P, 512], mybir.dt.float16,
                               name=f"wcc{kt}", tag=f"wcc{kt}")
                nc.scalar.dma_start(t[:], mcc[kt * P:(kt + 1) * P, :])
                wcc.append(t)
            for kt in range(4):
                t = wpool.tile([P, 512], mybir.dt.float16,
                               name=f"wcv{kt}", tag=f"wcv{kt}")
                nc.scalar.dma_start(t[:], mcv[kt * P:(kt + 1) * P, :])
                wcv.append(t)
            for kt in range(8):
                t = wpool.tile([P, 1024], mybir.dt.float16,
                               name=f"wn{kt}", tag=f"wn{kt}")
                e = nc.scalar if kt % 2 == 0 else nc.sync
                e.dma_start(t[:], mn[kt * P:(kt + 1) * P, :])
                wn.append(t)

            pools = (xpool, upool, fpool, spool, tpool, opool, pspool)
            if repeat > 1:
                with tc.For_i(0, repeat, 1):
                    _emit_body(nc, pools, (wcc, wcv, wn), xT, yT)
            else:
                _emit_body(nc, pools, (wcc, wcv, wn), xT, yT)

    nc.compile()
    return nc


def kernel(x: np.ndarray, W_first_col: np.ndarray, W_second_col: np.ndarray) -> np.ndarray:
    from concourse import bass_utils

    in_maps = make_in_maps(x, W_first_col, W_second_col)
    if "nc" not in _cache:
        _cache["nc"] = _build()
    try:
        res = bass_utils.run_bass_kernel_spmd(
            _cache["nc"], in_maps, core_ids=list(range(N_CORES))
        )
    except Exception:
        # transient device/exec failures usually clear on a retry
        res = bass_utils.run_bass_kernel_spmd(
            _cache["nc"], in_maps, core_ids=list(range(N_CORES))
        )
    out = np.concatenate(
        [np.ascontiguousarray(r["yT"].T) for r in res.results], axis=0)
    return out.reshape(8, 64, 64, OUT_DIM).astype(np.float32)
